# revision 1
# baseline (speedup 1.0000x reference)
"""Trainium2 Bass kernel for nn_Bert4Re: per-sample head-token gather, span
mean-pool, and all-ordered-pairs concat.

Strategy (data-parallel over batch, 4 samples per NeuronCore x 8 cores):
  - Host folds the integer index math (head_indexes gather + span windows)
    into a tiny per-sample pooling matrix WT[L, M] with
    WT[l, m] = (# t in [start_m, end_m) with head_indexes[t] == l) / len_m,
    so on device  pooled[m, :] = (WT.T @ seq)[m, :]  via TensorE.
  - The pair tensor (row (i,j) = [pooled_i | pooled_j]) is written straight
    from SBUF with two strided replication DMAs per sample, including the
    i==j diagonal rows (4.2% extra bytes buys perfectly regular descriptors).
  - Host drops the diagonal rows while copying per-core results into the
    final buffer (that gather copy is needed anyway) and computes the tiny
    integer `pairs` output directly.
"""

import os
import sys

import numpy as np

try:  # the axon site dir provides concourse on PYTHONPATH in this container
    import concourse.bass as bass  # noqa: F401
except ImportError:  # pragma: no cover - fallback for stripped environments
    sys.path.insert(0, "/opt/trn_rl_repo")
    import concourse.bass as bass  # noqa: F401

import concourse.tile as tile
from concourse import bacc, mybir
from concourse.bass_utils import run_bass_kernel_spmd

B, L, D, M = 32, 512, 768, 24
N_CORES = 8
BPC = B // N_CORES  # samples per core
GRID = M * M  # 576 pair rows per sample incl. diagonal
P = M * M - M  # 552 off-diagonal pair rows per sample
_PART = 128
_C = L // _PART  # K-chunks per matmul

_STATE: dict = {}


def build_nc(repeat: int = 1):
    """Build + compile the per-core Bass module.

    repeat > 1 unrolls the whole body N times (same I/O) — used only by the
    timing harness to measure per-iteration device time by differencing.
    """
    DT = mybir.dt.float32
    nc = bacc.Bacc("TRN2", target_bir_lowering=False, debug=False)
    seq_d = nc.dram_tensor("seq", [BPC, L, D], DT, kind="ExternalInput")
    wt_d = nc.dram_tensor("wt", [BPC, L, M], DT, kind="ExternalInput")
    out_d = nc.dram_tensor("out", [BPC, GRID, 2 * D], DT, kind="ExternalOutput")

    with tile.TileContext(nc) as tc:
        with (
            tc.tile_pool(name="io", bufs=3) as io_pool,
            tc.tile_pool(name="ps", bufs=2, space="PSUM") as psum_pool,
            tc.tile_pool(name="pl", bufs=3) as pl_pool,
        ):
            for r in range(repeat):
                for b in range(BPC):
                    stile = io_pool.tile([_PART, _C * D], DT, tag="stile")
                    nc.scalar.dma_start(
                        stile[:].rearrange("p (c d) -> p c d", c=_C),
                        seq_d[b].rearrange("(c p) d -> p c d", p=_PART),
                    )
                    wtile = io_pool.tile([_PART, _C * M], DT, tag="wtile")
                    nc.scalar.dma_start(
                        wtile[:].rearrange("p (c m) -> p c m", c=_C),
                        wt_d[b].rearrange("(c p) m -> p c m", p=_PART),
                    )

                    ps = psum_pool.tile([M, D], DT)
                    for c in range(_C):
                        for n0, nsz in ((0, 512), (512, 256)):
                            nc.tensor.matmul(
                                ps[:, n0 : n0 + nsz],
                                wtile[:, c * M : (c + 1) * M],
                                stile[:, c * D + n0 : c * D + n0 + nsz],
                                start=(c == 0),
                                stop=(c == _C - 1),
                            )
                    pooled = pl_pool.tile([M, D], DT)
                    nc.vector.tensor_copy(pooled[:], ps[:])

                    out3 = out_d[b].rearrange("(i j) d -> i j d", j=M)
                    src = pooled[:, :].unsqueeze(1).broadcast_to([M, M, D])
                    # obj half: out3[i, j, 0:D] = pooled[i]
                    nc.sync.dma_start(out3[:, :, 0:D], src)
                    # sub half: out3[i, j, D:2D] = pooled[j] (iterate j outer)
                    nc.sync.dma_start(out3[:, :, D : 2 * D].transpose([1, 0, 2]), src)
    nc.compile()
    return nc


def _get_nc():
    if "nc" not in _STATE:
        try:
            import jax

            cache_dir = os.environ.get(
                "BERT4RE_JAX_CACHE", "/tmp/jax_cache_bert4re"
            )
            jax.config.update("jax_compilation_cache_dir", cache_dir)
            jax.config.update("jax_persistent_cache_min_compile_time_secs", 0.0)
            jax.config.update("jax_persistent_cache_min_entry_size_bytes", 0)
        except Exception:
            pass
        _STATE["nc"] = build_nc()
    return _STATE["nc"]


def build_wt(head_indexes: np.ndarray, entity_mentions: np.ndarray) -> np.ndarray:
    """Fold gather + span mean-pool into WT[b, l, m] (f32) so that
    pooled[b] = WT[b].T @ sequence_output[b]."""
    head_indexes = np.asarray(head_indexes)
    entity_mentions = np.asarray(entity_mentions)
    b_, l_ = head_indexes.shape
    m_ = entity_mentions.shape[1]
    starts = entity_mentions[..., 0].astype(np.int64)  # [B, M]
    ends = entity_mentions[..., 1].astype(np.int64)
    lens = ends - starts  # >= 1
    kmax = int(lens.max())
    k = np.arange(kmax)[None, None, :]  # [1, 1, K]
    t = starts[..., None] + k  # [B, M, K]
    valid = k < lens[..., None]
    t = np.minimum(t, l_ - 1)
    bb = np.broadcast_to(np.arange(b_)[:, None, None], (b_, m_, kmax))
    mm = np.broadcast_to(np.arange(m_)[None, :, None], (b_, m_, kmax))
    ll = head_indexes[bb, t]  # gathered column index per (b, m, k)
    w = (valid / lens[..., None]).astype(np.float32)
    a = np.zeros((b_, m_, l_), np.float32)
    np.add.at(a, (bb, mm, ll), w)
    return np.ascontiguousarray(a.transpose(0, 2, 1))  # [B, L, M]


def _make_pairs(entity_mentions: np.ndarray) -> np.ndarray:
    em = np.asarray(entity_mentions)
    starts, ends = em[..., 0], em[..., 1]
    ii, jj = np.meshgrid(np.arange(M), np.arange(M), indexing="ij")
    off = ii != jj
    ii, jj = ii[off], jj[off]  # [P]
    idt = np.result_type(np.int32, em.dtype)
    batch_idx = np.repeat(np.arange(B), P)
    pairs = np.stack(
        [
            batch_idx.astype(idt),
            starts[:, ii].reshape(-1).astype(idt),
            ends[:, ii].reshape(-1).astype(idt),
            starts[:, jj].reshape(-1).astype(idt),
            ends[:, jj].reshape(-1).astype(idt),
        ],
        axis=-1,
    )
    return pairs


def kernel(sequence_output, head_indexes, entity_mentions):
    seq = np.ascontiguousarray(np.asarray(sequence_output, dtype=np.float32))
    head_indexes = np.asarray(head_indexes)
    entity_mentions = np.asarray(entity_mentions)
    assert seq.shape == (B, L, D), seq.shape

    wt = build_wt(head_indexes, entity_mentions)  # [B, L, M] f32

    nc = _get_nc()
    in_maps = [
        {"seq": seq[c * BPC : (c + 1) * BPC], "wt": wt[c * BPC : (c + 1) * BPC]}
        for c in range(N_CORES)
    ]
    res = run_bass_kernel_spmd(nc, in_maps, core_ids=list(range(N_CORES)))

    rel = np.empty((B, P, 2 * D), np.float32)
    for c in range(N_CORES):
        grid = np.asarray(res.results[c]["out"])  # [BPC, 576, 2D]
        # drop diagonal rows (q = 25*i) without fancy indexing:
        # rows [0..574] -> [23, 25] -> drop col 0
        v = grid[:, : GRID - 1, :].reshape(BPC, M - 1, M + 1, 2 * D)[:, :, 1:, :]
        rel[c * BPC : (c + 1) * BPC] = v.reshape(BPC, P, 2 * D)

    relation_predictions = rel.reshape(B * P, 2 * D)
    pairs = _make_pairs(entity_mentions)
    return relation_predictions, pairs


# revision 2
# speedup vs baseline: 1.7533x; 1.7533x over previous
"""Trainium2 Bass kernel for nn_Bert4Re: per-sample head-token gather, span
mean-pool, and all-ordered-pairs concat.

Strategy (data-parallel over batch, 4 samples per NeuronCore x 8 cores):
  - Host folds the integer index math (head_indexes gather + span windows)
    into a tiny per-sample pooling matrix WT[L, M] with
    WT[l, m] = (# t in [start_m, end_m) with head_indexes[t] == l) / len_m,
    so on device  pooled[m, :] = (WT.T @ seq)[m, :]  via TensorE.
  - The pair tensor (row (i,j) = [pooled_i | pooled_j]) is written straight
    from SBUF with two strided replication DMAs per sample, including the
    i==j diagonal rows (4.2% extra bytes buys perfectly regular descriptors).
  - Host drops the diagonal rows while copying per-core results into the
    final buffer (that gather copy is needed anyway) and computes the tiny
    integer `pairs` output directly.
"""

import os
import sys

import numpy as np

try:  # the axon site dir provides concourse on PYTHONPATH in this container
    import concourse.bass as bass  # noqa: F401
except ImportError:  # pragma: no cover - fallback for stripped environments
    sys.path.insert(0, "/opt/trn_rl_repo")
    import concourse.bass as bass  # noqa: F401

import concourse.tile as tile
from concourse import bacc, mybir
from concourse.bass_utils import run_bass_kernel_spmd

B, L, D, M = 32, 512, 768, 24
N_CORES = 8
BPC = B // N_CORES  # samples per core
GRID = M * M  # 576 pair rows per sample incl. diagonal
P = M * M - M  # 552 off-diagonal pair rows per sample
_PART = 128
_C = L // _PART  # K-chunks per matmul

_STATE: dict = {}


def build_nc(repeat: int = 1):
    """Build + compile the per-core Bass module.

    repeat > 1 unrolls the whole body N times (same I/O) — used only by the
    timing harness to measure per-iteration device time by differencing.
    """
    DT = mybir.dt.float32
    nc = bacc.Bacc("TRN2", target_bir_lowering=False, debug=False)
    seq_d = nc.dram_tensor("seq", [BPC, L, D], DT, kind="ExternalInput")
    wt_d = nc.dram_tensor("wt", [BPC, L, M], DT, kind="ExternalInput")
    out_d = nc.dram_tensor("out", [BPC, GRID, 2 * D], DT, kind="ExternalOutput")

    with tile.TileContext(nc) as tc:
        with (
            tc.tile_pool(name="io", bufs=3) as io_pool,
            tc.tile_pool(name="ps", bufs=2, space="PSUM") as psum_pool,
            tc.tile_pool(name="pl", bufs=3) as pl_pool,
        ):
            if repeat == 0:  # timing baseline: minimal NEFF (one tiny DMA)
                t0 = io_pool.tile([1, 128], DT, tag="tiny")
                nc.sync.dma_start(t0[:], seq_d[0, 0:1, 0:128])
                nc.sync.dma_start(out_d[0, 0:1, 0:128], t0[:])
            for r in range(repeat):
                for b in range(BPC):
                    stile = io_pool.tile([_PART, _C * D], DT, tag="stile")
                    nc.scalar.dma_start(
                        stile[:].rearrange("p (c d) -> p c d", c=_C),
                        seq_d[b].rearrange("(c p) d -> p c d", p=_PART),
                    )
                    wtile = io_pool.tile([_PART, _C * M], DT, tag="wtile")
                    nc.scalar.dma_start(
                        wtile[:].rearrange("p (c m) -> p c m", c=_C),
                        wt_d[b].rearrange("(c p) m -> p c m", p=_PART),
                    )

                    ps = psum_pool.tile([M, D], DT)
                    for c in range(_C):
                        for n0, nsz in ((0, 512), (512, 256)):
                            nc.tensor.matmul(
                                ps[:, n0 : n0 + nsz],
                                wtile[:, c * M : (c + 1) * M],
                                stile[:, c * D + n0 : c * D + n0 + nsz],
                                start=(c == 0),
                                stop=(c == _C - 1),
                            )
                    pooled = pl_pool.tile([M, D], DT)
                    nc.vector.tensor_copy(pooled[:], ps[:])

                    out3 = out_d[b].rearrange("(i j) d -> i j d", j=M)
                    src = pooled[:, :].unsqueeze(1).broadcast_to([M, M, D])
                    # obj half: out3[i, j, 0:D] = pooled[i]
                    nc.sync.dma_start(out3[:, :, 0:D], src)
                    # sub half: out3[i, j, D:2D] = pooled[j] (iterate j outer)
                    nc.sync.dma_start(out3[:, :, D : 2 * D].transpose([1, 0, 2]), src)
    nc.compile()
    return nc


def _get_nc():
    if "nc" not in _STATE:
        try:
            import jax

            cache_dir = os.environ.get(
                "BERT4RE_JAX_CACHE", "/tmp/jax_cache_bert4re"
            )
            jax.config.update("jax_compilation_cache_dir", cache_dir)
            jax.config.update("jax_persistent_cache_min_compile_time_secs", 0.0)
            jax.config.update("jax_persistent_cache_min_entry_size_bytes", 0)
        except Exception:
            pass
        _STATE["nc"] = build_nc()
    return _STATE["nc"]


def build_wt(head_indexes: np.ndarray, entity_mentions: np.ndarray) -> np.ndarray:
    """Fold gather + span mean-pool into WT[b, l, m] (f32) so that
    pooled[b] = WT[b].T @ sequence_output[b]."""
    head_indexes = np.asarray(head_indexes)
    entity_mentions = np.asarray(entity_mentions)
    b_, l_ = head_indexes.shape
    m_ = entity_mentions.shape[1]
    starts = entity_mentions[..., 0].astype(np.int64)  # [B, M]
    ends = entity_mentions[..., 1].astype(np.int64)
    lens = ends - starts  # >= 1
    kmax = int(lens.max())
    k = np.arange(kmax)[None, None, :]  # [1, 1, K]
    t = starts[..., None] + k  # [B, M, K]
    valid = k < lens[..., None]
    t = np.minimum(t, l_ - 1)
    bb = np.broadcast_to(np.arange(b_)[:, None, None], (b_, m_, kmax))
    mm = np.broadcast_to(np.arange(m_)[None, :, None], (b_, m_, kmax))
    ll = head_indexes[bb, t]  # gathered column index per (b, m, k)
    w = (valid / lens[..., None]).astype(np.float32)
    a = np.zeros((b_, m_, l_), np.float32)
    np.add.at(a, (bb, mm, ll), w)
    return np.ascontiguousarray(a.transpose(0, 2, 1))  # [B, L, M]


def _make_pairs(entity_mentions: np.ndarray) -> np.ndarray:
    em = np.asarray(entity_mentions)
    starts, ends = em[..., 0], em[..., 1]
    ii, jj = np.meshgrid(np.arange(M), np.arange(M), indexing="ij")
    off = ii != jj
    ii, jj = ii[off], jj[off]  # [P]
    idt = np.result_type(np.int32, em.dtype)
    batch_idx = np.repeat(np.arange(B), P)
    pairs = np.stack(
        [
            batch_idx.astype(idt),
            starts[:, ii].reshape(-1).astype(idt),
            ends[:, ii].reshape(-1).astype(idt),
            starts[:, jj].reshape(-1).astype(idt),
            ends[:, jj].reshape(-1).astype(idt),
        ],
        axis=-1,
    )
    return pairs


def kernel(sequence_output, head_indexes, entity_mentions):
    seq = np.ascontiguousarray(np.asarray(sequence_output, dtype=np.float32))
    head_indexes = np.asarray(head_indexes)
    entity_mentions = np.asarray(entity_mentions)
    assert seq.shape == (B, L, D), seq.shape

    wt = build_wt(head_indexes, entity_mentions)  # [B, L, M] f32

    nc = _get_nc()
    in_maps = [
        {"seq": seq[c * BPC : (c + 1) * BPC], "wt": wt[c * BPC : (c + 1) * BPC]}
        for c in range(N_CORES)
    ]
    res = run_bass_kernel_spmd(nc, in_maps, core_ids=list(range(N_CORES)))

    rel = np.empty((B, P, 2 * D), np.float32)
    for c in range(N_CORES):
        grid = np.asarray(res.results[c]["out"])  # [BPC, 576, 2D]
        # drop diagonal rows (q = 25*i) without fancy indexing:
        # rows [0..574] -> [23, 25] -> drop col 0
        v = grid[:, : GRID - 1, :].reshape(BPC, M - 1, M + 1, 2 * D)[:, :, 1:, :]
        rel[c * BPC : (c + 1) * BPC] = v.reshape(BPC, P, 2 * D)

    relation_predictions = rel.reshape(B * P, 2 * D)
    pairs = _make_pairs(entity_mentions)
    return relation_predictions, pairs


# revision 14
# speedup vs baseline: 4.1607x; 2.3731x over previous
"""Trainium2 Bass kernel for nn_Bert4Re: per-sample head-token gather, span
mean-pool, and all-ordered-pairs concat.

Strategy (data-parallel over batch, 4 samples per NeuronCore x 8 cores):
  - Host folds the integer index math (head_indexes gather + span windows)
    into a tiny per-sample pooling matrix WT[L, M] with
    WT[l, m] = (# t in [start_m, end_m) with head_indexes[t] == l) / len_m,
    so on device  pooled[m, :] = (WT.T @ seq)[m, :]  via TensorE.
  - The pair tensor (row (i,j) = [pooled_i | pooled_j]) is written straight
    from SBUF with two strided replication DMAs per sample, including the
    i==j diagonal rows (4.2% extra bytes buys perfectly regular descriptors).
  - Host drops the diagonal rows while copying per-core results into the
    final buffer (that gather copy is needed anyway) and computes the tiny
    integer `pairs` output directly.
"""

import os
import sys

import numpy as np

try:  # the axon site dir provides concourse on PYTHONPATH in this container
    import concourse.bass as bass  # noqa: F401
except ImportError:  # pragma: no cover - fallback for stripped environments
    sys.path.insert(0, "/opt/trn_rl_repo")
    import concourse.bass as bass  # noqa: F401

import concourse.tile as tile
from concourse import bacc, mybir
from concourse.bass_utils import run_bass_kernel_spmd

B, L, D, M = 32, 512, 768, 24
MAX_SPAN = 16
N_CORES = 8
BPC = B // N_CORES  # samples per core
GRID = M * M  # 576 pair rows per sample incl. diagonal
P = M * M - M  # 552 off-diagonal pair rows per sample
_PART = 128
_C = L // _PART  # K-chunks (dense path)
KPAD = M * MAX_SPAN  # 384 gather slots (gather path)
_CG = KPAD // _PART  # 3 K-chunks (gather path)
OOB_SENTINEL = 99999  # > BPC*L-1 -> indirect DMA skips the row

_STATE: dict = {}


def build_nc(repeat: int = 1, gather: bool = True):
    """Build + compile the per-core Bass module.

    repeat > 1 unrolls the whole body N times (same I/O) — used only by the
    timing harness to measure per-iteration device time by differencing.
    """
    DT = mybir.dt.float32
    DTI = mybir.dt.int32
    nK = _CG if gather else _C
    nc = bacc.Bacc("TRN2", target_bir_lowering=False, debug=False)
    seq_d = nc.dram_tensor("seq", [BPC, L, D], DT, kind="ExternalInput")
    if gather:
        wt_d = nc.dram_tensor("wt", [BPC, KPAD, M], DT, kind="ExternalInput")
        idx_d = nc.dram_tensor("idx", [BPC, KPAD], DTI, kind="ExternalInput")
    else:
        wt_d = nc.dram_tensor("wt", [BPC, L, M], DT, kind="ExternalInput")
        idx_d = None
    out_d = nc.dram_tensor("out", [BPC, GRID, 2 * D], DT, kind="ExternalOutput")

    NGB = 3 * _CG  # gather buffers: triple-buffered per K-chunk
    with tile.TileContext(nc) as tc:
        with (
            tc.tile_pool(name="io", bufs=3) as io_pool,
            tc.tile_pool(name="gb", bufs=1) as gb_pool,
            tc.tile_pool(name="ps", bufs=2, space="PSUM") as psum_pool,
            tc.tile_pool(name="pl", bufs=3) as pl_pool,
        ):
            if repeat == 0:  # timing baseline: minimal NEFF (one tiny DMA)
                t0 = io_pool.tile([1, 128], DT, tag="tiny")
                nc.sync.dma_start(t0[:], seq_d[0, 0:1, 0:128])
                nc.sync.dma_start(out_d[0, 0:1, 0:128], t0[:])
            gbufs = []
            if gather and repeat > 0:
                # persistent gather buffers, zeroed once: OOB-skipped pad
                # slots leave old contents, which the matmul multiplies by 0
                # weight — initial SBUF garbage could be NaN, so zero it.
                for i in range(NGB):
                    gt = gb_pool.tile([_PART, D], DT, tag=f"gbuf{i}")
                    nc.vector.memset(gt[:], 0.0)
                    gbufs.append(gt)
            for r in range(repeat):
                for b in range(BPC):
                    wtile = io_pool.tile([_PART, nK * M], DT, tag="wtile")
                    nc.scalar.dma_start(
                        wtile[:].rearrange("p (c m) -> p c m", c=nK),
                        wt_d[b].rearrange("(c p) m -> p c m", p=_PART),
                    )
                    chunks = []
                    if gather:
                        # idx[b] laid out [c, p] on host -> SBUF [p, c]
                        itile = io_pool.tile([_PART, _CG], DTI, tag="itile")
                        nc.scalar.dma_start(
                            itile[:],
                            idx_d[b].rearrange("(c p) -> p c", p=_PART),
                        )
                        it = r * BPC + b
                        for c in range(_CG):
                            ch = gbufs[(it * _CG + c) % NGB]
                            nc.gpsimd.indirect_dma_start(
                                out=ch[:],
                                out_offset=None,
                                in_=seq_d[:],
                                in_offset=bass.IndirectOffsetOnAxis(
                                    ap=itile[:, c : c + 1], axis=1
                                ),
                                bounds_check=BPC * L - 1,
                                oob_is_err=False,
                            )
                            chunks.append(ch)
                    else:
                        # per-K-chunk loads so matmul c overlaps load c+1
                        for c in range(_C):
                            ch = io_pool.tile([_PART, D], DT, tag=f"seq{c}")
                            nc.scalar.dma_start(
                                ch[:], seq_d[b, c * _PART : (c + 1) * _PART, :]
                            )
                            chunks.append(ch)

                    ps = psum_pool.tile([M, D], DT)
                    for c in range(nK):
                        for n0, nsz in ((0, 512), (512, 256)):
                            nc.tensor.matmul(
                                ps[:, n0 : n0 + nsz],
                                wtile[:, c * M : (c + 1) * M],
                                chunks[c][:, n0 : n0 + nsz],
                                start=(c == 0),
                                stop=(c == nK - 1),
                            )
                    pooled = pl_pool.tile([M, D], DT)
                    nc.vector.tensor_copy(pooled[:], ps[:])

                    out3 = out_d[b].rearrange("(i j) d -> i j d", j=M)
                    src = pooled[:, :].unsqueeze(1).broadcast_to([M, M, D])
                    # obj half: out3[i, j, 0:D] = pooled[i]
                    nc.sync.dma_start(out3[:, :, 0:D], src)
                    # sub half: out3[i, j, D:2D] = pooled[j] (iterate j outer)
                    nc.sync.dma_start(out3[:, :, D : 2 * D].transpose([1, 0, 2]), src)
    nc.compile()
    return nc


def _get_nc():
    if "nc" not in _STATE:
        try:
            import jax

            cache_dir = os.environ.get(
                "BERT4RE_JAX_CACHE", "/tmp/jax_cache_bert4re"
            )
            jax.config.update("jax_compilation_cache_dir", cache_dir)
            jax.config.update("jax_persistent_cache_min_compile_time_secs", 0.0)
            jax.config.update("jax_persistent_cache_min_entry_size_bytes", 0)
        except Exception:
            pass
        _STATE["nc"] = build_nc()
    return _STATE["nc"]


def build_wt(head_indexes: np.ndarray, entity_mentions: np.ndarray) -> np.ndarray:
    """Fold gather + span mean-pool into WT[b, l, m] (f32) so that
    pooled[b] = WT[b].T @ sequence_output[b]."""
    head_indexes = np.asarray(head_indexes)
    entity_mentions = np.asarray(entity_mentions)
    b_, l_ = head_indexes.shape
    m_ = entity_mentions.shape[1]
    starts = entity_mentions[..., 0].astype(np.int64)  # [B, M]
    ends = entity_mentions[..., 1].astype(np.int64)
    lens = ends - starts  # >= 1
    kmax = int(lens.max())
    k = np.arange(kmax)[None, None, :]  # [1, 1, K]
    t = starts[..., None] + k  # [B, M, K]
    valid = k < lens[..., None]
    t = np.minimum(t, l_ - 1)
    bb = np.broadcast_to(np.arange(b_)[:, None, None], (b_, m_, kmax))
    mm = np.broadcast_to(np.arange(m_)[None, :, None], (b_, m_, kmax))
    ll = head_indexes[bb, t]  # gathered column index per (b, m, k)
    w = (valid / lens[..., None]).astype(np.float32)
    a = np.zeros((b_, m_, l_), np.float32)
    np.add.at(a, (bb, mm, ll), w)
    return np.ascontiguousarray(a.transpose(0, 2, 1))  # [B, L, M]


def build_gather_inputs(head_indexes, entity_mentions):
    """Gather-path host prep: per-slot DRAM row indices (within the per-core
    shard) + the slot->mention averaging matrix.

    Slot k = m*MAX_SPAN + j holds span token j of mention m:
      idx[b, k] = (b % BPC)*L + head_indexes[b, starts[m]+j]   (pad -> row 0)
      wt[b, k, m'] = (m' == m) * (j < len_m) / len_m
    """
    head = np.asarray(head_indexes).astype(np.int64)
    em = np.asarray(entity_mentions)
    starts = em[..., 0].astype(np.int64)  # [B, M]
    lens = (em[..., 1] - em[..., 0]).astype(np.int64)
    j = np.arange(MAX_SPAN)[None, None, :]
    valid = j < lens[..., None]  # [B, M, S]
    t = np.minimum(starts[..., None] + j, L - 1)
    bb = np.broadcast_to(np.arange(B)[:, None, None], t.shape)
    shard_b = (np.arange(B) % BPC)[:, None, None]
    # pad slots get an out-of-bounds sentinel: the DMA skips them entirely
    # (no bytes moved); their 0 weight kills the stale (pre-zeroed) data.
    rows = np.where(valid, head[bb, t] + shard_b * L, OOB_SENTINEL)  # [B, M, S]
    idx = rows.reshape(B, KPAD).astype(np.int32)

    w = (valid / lens[..., None]).astype(np.float32)  # [B, M, S]
    wtg4 = np.zeros((B, M, MAX_SPAN, M), np.float32)
    mi = np.arange(M)
    wtg4[:, mi, :, mi] = w.transpose(1, 0, 2)
    wtg = wtg4.reshape(B, KPAD, M)
    return idx, wtg


def _make_pairs(entity_mentions: np.ndarray) -> np.ndarray:
    em = np.asarray(entity_mentions)
    starts, ends = em[..., 0], em[..., 1]
    ii, jj = np.meshgrid(np.arange(M), np.arange(M), indexing="ij")
    off = ii != jj
    ii, jj = ii[off], jj[off]  # [P]
    idt = np.result_type(np.int32, em.dtype)
    batch_idx = np.repeat(np.arange(B), P)
    pairs = np.stack(
        [
            batch_idx.astype(idt),
            starts[:, ii].reshape(-1).astype(idt),
            ends[:, ii].reshape(-1).astype(idt),
            starts[:, jj].reshape(-1).astype(idt),
            ends[:, jj].reshape(-1).astype(idt),
        ],
        axis=-1,
    )
    return pairs


def make_in_maps(sequence_output, head_indexes, entity_mentions):
    seq = np.ascontiguousarray(np.asarray(sequence_output, dtype=np.float32))
    assert seq.shape == (B, L, D), seq.shape
    idx, wtg = build_gather_inputs(head_indexes, entity_mentions)
    return [
        {
            "seq": seq[c * BPC : (c + 1) * BPC],
            "wt": wtg[c * BPC : (c + 1) * BPC],
            "idx": idx[c * BPC : (c + 1) * BPC],
        }
        for c in range(N_CORES)
    ]


def kernel(sequence_output, head_indexes, entity_mentions):
    head_indexes = np.asarray(head_indexes)
    entity_mentions = np.asarray(entity_mentions)
    in_maps = make_in_maps(sequence_output, head_indexes, entity_mentions)
    nc = _get_nc()
    res = run_bass_kernel_spmd(nc, in_maps, core_ids=list(range(N_CORES)))

    rel = np.empty((B, P, 2 * D), np.float32)
    for c in range(N_CORES):
        grid = np.asarray(res.results[c]["out"])  # [BPC, 576, 2D]
        # drop diagonal rows (q = 25*i) without fancy indexing:
        # rows [0..574] -> [23, 25] -> drop col 0
        v = grid[:, : GRID - 1, :].reshape(BPC, M - 1, M + 1, 2 * D)[:, :, 1:, :]
        rel[c * BPC : (c + 1) * BPC] = v.reshape(BPC, P, 2 * D)

    relation_predictions = rel.reshape(B * P, 2 * D)
    pairs = _make_pairs(entity_mentions)
    return relation_predictions, pairs
